# revision 15
# baseline (speedup 1.0000x reference)
"""Binarized 4-layer MLP (8192x784 -> 6144 -> 6144 -> 6144 -> 10, log_softmax)
on 8 Trainium2 NeuronCores, data-parallel over the batch.

Per-core dataflow (batch slice of 1024, feature-major activations [feat, batch]):
  fc1: x @ sign(w1).T as a hybrid split of x: hi = fp16(x) plus 112 exact fp16
       lo rows in the 7th k-tile's padding, and the remaining 672 lo rows
       (lo = x - hi) scaled by 2^9 in fp8e4 via 3 DoubleRow blocks whose
       stationary weights are +-2^-9 (exactly representable; PE handles fp8
       subnormal weights losslessly - HW verified). All terms accumulate into
       one PSUM group, so fc1 costs 10 passes instead of 13 at ~2^-15
       relative x error, which flips only ~1e-4 of h1 signs.
  fc2/fc3: sign(h) @ sign(w).T in fp8e4 with DoubleRow perf mode (fc2 uses
       the SwInterleave stationary layout; measured identical to DoubleRow).
       All products are +-1 and partial sums are small integers, so fp32 PSUM
       accumulation is bit-exact regardless of order.
  fc4: fused into the fc3 m-loop, single fp16 pass (w4 and h3 in fp16).
  log_softmax: PE-transpose of the logits to [batch, 10] tiles, exp/sum/ln
       without max-subtraction (logits are O(1), no overflow risk).

Schedule notes:
  - startup DMAs are split small and spread over the sync/gpsimd (+scalar
    early) DGE queues in consumption order
  - h1/h2 are split into lo/hi tiles so the next layer's first matmul
    doesn't wait on the last sign() of the previous layer
  - fc3/fc4 run per batch-half; the softmax tail of half 0 hides under the
    fc3 matmuls of half 1; one Ln at the very end serves both halves
"""

import numpy as np
import ml_dtypes

import concourse.bass as bass
import concourse.mybir as mybir
from concourse import bacc
from concourse.tile import TileContext
from concourse.bass_utils import run_bass_kernel_spmd

dt = mybir.dt

CORES = 8
B = 8192
BC = B // CORES          # 1024 batch rows per core
DIN = 784
K16 = 7                  # fc1 fp16 k-tiles (784 hi + 112 exact lo rows)
KD = 3                   # fc1 fp8 DoubleRow blocks (672 lo rows + 96 pad)
NLO = 672                # lo rows carried in fp8
LSC = 512.0              # lo scale 2^9 (weights +-2^-9)
DH = 6144
MT = DH // 128           # 48 feature tiles
MH = MT // 2             # 24 tiles per lo/hi activation buffer
KB = DH // 256           # 24 DoubleRow contraction blocks
DOUT = 10
NH = BC // 512           # 2 moving halves of 512
NJ2 = 512 // 128         # 4 output j-tiles per half
MQ = 12                  # fc1 m-groups (w1 streamed per 4 m-tiles)
MPQ = MT // MQ

BF16 = ml_dtypes.bfloat16
FP8 = mybir.dt.np(dt.float8e4)

last_exec_time_ns = None


def _build_program():
    nc = bacc.Bacc("TRN2", target_bir_lowering=False, debug=False,
                   num_devices=CORES)

    xt16 = nc.dram_tensor("xt16", [128, K16, BC], dt.float16,
                          kind="ExternalInput").ap()
    xt8 = nc.dram_tensor("xt8", [128, KD, 2, BC], dt.float8e4,
                         kind="ExternalInput").ap()
    w1t16 = nc.dram_tensor("w1t16", [MQ, 128, K16, MPQ * 128], dt.float16,
                           kind="ExternalInput").ap()
    w1t8 = nc.dram_tensor("w1t8", [MQ, 128, KD, 2, MPQ * 128], dt.float8e4,
                          kind="ExternalInput").ap()
    w2p = nc.dram_tensor("w2p", [MT, 128, KB, 256], dt.float8e4,
                         kind="ExternalInput").ap()
    w3p = nc.dram_tensor("w3p", [MT, 128, KB, 2, 128], dt.float8e4,
                         kind="ExternalInput").ap()
    w4p = nc.dram_tensor("w4p", [128, MT, DOUT], dt.float16,
                         kind="ExternalInput").ap()
    b1p = nc.dram_tensor("b1p", [128, MT], dt.float32, kind="ExternalInput").ap()
    b2p = nc.dram_tensor("b2p", [128, MT], dt.float32, kind="ExternalInput").ap()
    b3p = nc.dram_tensor("b3p", [128, MT], dt.float32, kind="ExternalInput").ap()
    b4p = nc.dram_tensor("b4p", [DOUT, 1], dt.float32, kind="ExternalInput").ap()
    identp = nc.dram_tensor("identp", [DOUT, DOUT], dt.float32,
                            kind="ExternalInput").ap()
    out = nc.dram_tensor("out", [BC, DOUT], dt.float32, kind="ExternalOutput").ap()

    DR = mybir.MatmulPerfMode.DoubleRow
    DRSW = mybir.MatmulPerfMode.DoubleRowSwInterleave
    AF = mybir.ActivationFunctionType

    with TileContext(nc) as tc:
        with tc.tile_pool(name="consts", bufs=1) as cpool, \
             tc.tile_pool(name="h1p", bufs=1) as h1pool:
            h1 = [h1pool.tile([128, MH, BC], dt.float8e4, tag=f"h1{i}",
                              name=f"h1{i}") for i in range(2)]

            # prefetched first w2/w3 m-tiles (their zone opens mid-program)
            w2f = cpool.tile([128, KB, 256], dt.float8e4)
            w3f = cpool.tile([128, KB, 2, 128], dt.float8e4)
            b1_sb = cpool.tile([128, MT], dt.float32)
            b2_sb = cpool.tile([128, MT], dt.float32)
            b3_sb = cpool.tile([128, MT], dt.float32)
            b4_sb = cpool.tile([DOUT, 1], dt.float32)
            w4_sb = cpool.tile([128, MT, DOUT], dt.float16)
            ident = cpool.tile([DOUT, DOUT], dt.float32)

            # ---------------- fc1 ----------------
            with tc.tile_pool(name="fc1in", bufs=1) as fpool, \
                 tc.tile_pool(name="w1pool", bufs=3) as w1pool, \
                 tc.tile_pool(name="ps1", bufs=3, space="PSUM") as ps1:
                # startup DMAs, small pieces in consumption order; scalar's
                # DGE only helps before the ACT engine starts sign() work
                jobs = []
                xt_half = {}
                w1q0a = {}
                w1q0b = {}
                tiles = {}
                for k in range(K16):
                    txa = fpool.tile([128, 512], dt.float16, tag=f"xta_{k}",
                                     name=f"xta_{k}")
                    txb = fpool.tile([128, 512], dt.float16, tag=f"xtb_{k}",
                                     name=f"xtb_{k}")
                    twa = fpool.tile([128, 128], dt.float16, tag=f"w1a_{k}",
                                     name=f"w1a_{k}")
                    twb = fpool.tile([128, 384], dt.float16, tag=f"w1b_{k}",
                                     name=f"w1b_{k}")
                    tiles[k] = (txa, txb, twa, twb)
                    xt_half[(k, 0)] = txa[:, :]
                    xt_half[(k, 1)] = txb[:, :]
                    w1q0a[k] = twa
                    w1q0b[k] = twb
                x8t = fpool.tile([128, KD, 2, BC], dt.float8e4)
                w1q08 = fpool.tile([128, KD, 2, MPQ * 128], dt.float8e4)

                def kjobs(k):
                    txa, txb, twa, twb = tiles[k]
                    return [(twa[:, :], w1t16[0, :, k, 0:128]),
                            (txa[:, :], xt16[:, k, 0:512]),
                            (txb[:, :], xt16[:, k, 512:1024]),
                            (twb[:, :], w1t16[0, :, k, 128:512])]

                # consumption order: k=0, then the sandwiched DR operands,
                # then k=1..6, then the m>0 parts of the q0 weights
                jobs += kjobs(0)
                jobs += [(w1q08[:, :, :, 0:128], w1t8[0, :, :, :, 0:128])]
                for b in range(KD):
                    jobs += [(x8t[:, b], xt8[:, b])]
                for k in range(1, K16):
                    jobs += kjobs(k)
                jobs += [(w1q08[:, :, :, 128:512], w1t8[0, :, :, :, 128:512])]
                for i, (dst, src) in enumerate(jobs):
                    q = ([nc.sync, nc.gpsimd, nc.scalar][i % 3] if i < 12
                         else [nc.sync, nc.gpsimd][i % 2])
                    q.dma_start(out=dst, in_=src)

                nc.sync.dma_start(out=b1_sb[:], in_=b1p[:])
                nc.gpsimd.dma_start(out=b2_sb[:], in_=b2p[:])
                nc.sync.dma_start(out=b3_sb[:], in_=b3p[:])
                nc.gpsimd.dma_start(out=b4_sb[:], in_=b4p[:])
                nc.sync.dma_start(out=w4_sb[:], in_=w4p[:])
                nc.sync.dma_start(out=ident[:], in_=identp[:])

                for q in range(MQ):
                    if q == 0:
                        def lhs16(k, mi):
                            if mi == 0:
                                return w1q0a[k][:, :]
                            return w1q0b[k][:, (mi - 1) * 128:mi * 128]

                        def lhs8(b, mi):
                            return w1q08[:, b, :, mi * 128:(mi + 1) * 128]
                    else:
                        # q==1 must load during the startup crunch (sync);
                        # later groups dispatch from the scalar queue, which
                        # is backed up behind sign() ACTs — a free just-in-
                        # time delay that keeps these 1.3MB streams out of
                        # the startup DMA crunch
                        wq = nc.sync if q == 1 else nc.scalar
                        w1q16 = w1pool.tile([128, K16, MPQ * 128], dt.float16,
                                            tag="w1q16")
                        wq.dma_start(out=w1q16[:], in_=w1t16[q])
                        w1q8 = w1pool.tile([128, KD, 2, MPQ * 128],
                                           dt.float8e4, tag="w1q8")
                        (nc.gpsimd if q == 1 else nc.scalar).dma_start(
                            out=w1q8[:], in_=w1t8[q])

                        def lhs16(k, mi, w1q16=w1q16):
                            return w1q16[:, k, mi * 128:(mi + 1) * 128]

                        def lhs8(b, mi, w1q8=w1q8):
                            return w1q8[:, b, :, mi * 128:(mi + 1) * 128]
                    for mi in range(MPQ):
                        m = q * MPQ + mi
                        psum = ps1.tile([128, BC], dt.float32, tag="ps1")
                        # DR blocks sandwiched mid-group: a DR matmul at an
                        # accumulation-group boundary costs an extra ~200ns
                        # (unpipelined LDWEIGHTS); fp16 edges don't
                        for n in range(NH):
                            nc.tensor.matmul(
                                psum[:, n * 512:(n + 1) * 512],
                                lhs16(0, mi),
                                xt_half[(0, n)],
                                start=True,
                                stop=False,
                            )
                        for b in range(KD):
                            for n in range(NH):
                                nc.tensor.matmul(
                                    psum[:, n * 512:(n + 1) * 512],
                                    lhs8(b, mi),
                                    x8t[:, b, :, n * 512:(n + 1) * 512],
                                    start=False,
                                    stop=False,
                                    perf_mode=DR,
                                )
                        for k in range(1, K16):
                            for n in range(NH):
                                nc.tensor.matmul(
                                    psum[:, n * 512:(n + 1) * 512],
                                    lhs16(k, mi),
                                    xt_half[(k, n)],
                                    start=False,
                                    stop=(k == K16 - 1),
                                )
                        nc.scalar.sign(h1[m // MH][:, m % MH, :], psum[:, :],
                                       bias=b1_sb[:, m:m + 1])
                        if m == 20:
                            # deferred w2/w3 first-tile prefetch: scalar's
                            # queue is behind ~20 signs, so these 786KB
                            # loads dispatch ~100us in, clear of the crunch
                            nc.scalar.dma_start(out=w2f[:], in_=w2p[0])
                        elif m == 21:
                            nc.scalar.dma_start(out=w3f[:], in_=w3p[0])

            # ---------------- fc2 ----------------
            def pair(h, b, n):
                # moving [128, 2, 512] for DR block b out of lo/hi buffers
                t = 2 * b
                return h[t // MH][:, t % MH:t % MH + 2, n * 512:(n + 1) * 512]

            with tc.tile_pool(name="h2p", bufs=1) as h2pool:
                h2 = [h2pool.tile([128, MH, BC], dt.float8e4, tag=f"h2{i}",
                                  name=f"h2{i}") for i in range(2)]
                with tc.tile_pool(name="w2pool", bufs=3) as w2pool, \
                     tc.tile_pool(name="ps2", bufs=3, space="PSUM") as ps2:
                    for m in range(MT):
                        if m == 0:
                            wsb = w2f
                        else:
                            wsb = w2pool.tile([128, KB, 256], dt.float8e4,
                                              tag="w2")
                            nc.sync.dma_start(out=wsb[:], in_=w2p[m])
                        psum = ps2.tile([128, BC], dt.float32, tag="ps2")
                        for n in range(NH):
                            for b in range(KB):
                                nc.tensor.matmul(
                                    psum[:, n * 512:(n + 1) * 512],
                                    wsb[:, b],
                                    pair(h1, b, n),
                                    start=(b == 0),
                                    stop=(b == KB - 1),
                                    perf_mode=DRSW,
                                )
                        nc.scalar.sign(h2[m // MH][:, m % MH, :], psum[:, :],
                                       bias=b2_sb[:, m:m + 1])

                # ------------- fc3 + fused fc4 + log_softmax -------------
                with tc.tile_pool(name="lgp", bufs=1, space="PSUM") as lgp, \
                     tc.tile_pool(name="tpp", bufs=2, space="PSUM") as tpp, \
                     tc.tile_pool(name="smp", bufs=1) as smp, \
                     tc.tile_pool(name="w3pool", bufs=3) as w3pool, \
                     tc.tile_pool(name="h3pool", bufs=18) as h3pool, \
                     tc.tile_pool(name="ps3", bufs=3, space="PSUM") as ps3:
                    zex = smp.tile([128, NH * NJ2, DOUT], dt.float32)
                    zlog = smp.tile([128, NH * NJ2, DOUT], dt.float32)
                    lg_psums = {}
                    lg_sbs = {}

                    def tail_head(n):
                        lg_sb = smp.tile([DOUT, 512], dt.float32,
                                         tag=f"lgsb{n}", name=f"lgsb{n}")
                        nc.scalar.activation(lg_sb[:], lg_psums[n][:],
                                             AF.Identity, bias=b4_sb[:, 0:1])
                        lg_sbs[n] = lg_sb

                    def tail_j(n, j):
                        t = n * NJ2 + j
                        tp = tpp.tile([128, DOUT], dt.float32, tag="tp",
                                      name="tp")
                        nc.tensor.transpose(
                            tp[:], lg_sbs[n][:, j * 128:(j + 1) * 128],
                            ident[:])
                        nc.scalar.activation(zex[:, t, :], tp[:], AF.Exp)
                        nc.vector.tensor_scalar_add(zlog[:, t, :], tp[:], 0.0)

                    for n in range(NH):
                        lg_psum = lgp.tile([DOUT, 512], dt.float32,
                                           tag=f"lg{n}", name=f"lg{n}")
                        lg_psums[n] = lg_psum
                        h3_tiles = [None] * MT

                        def fc4_mm(m, lg_psum=lg_psum, h3_tiles=h3_tiles):
                            nc.tensor.matmul(
                                lg_psum[:, :],
                                w4_sb[:, m, :],
                                h3_tiles[m][:, :],
                                start=(m == 0),
                                stop=(m == MT - 1),
                            )

                        for m in range(MT):
                            if n == 0 and m == 0:
                                wsb = w3f
                            else:
                                wsb = w3pool.tile([128, KB, 2, 128],
                                                  dt.float8e4, tag="w3")
                                (nc.sync if m % 2 else nc.gpsimd).dma_start(
                                    out=wsb[:], in_=w3p[m])
                            psum = ps3.tile([128, 512], dt.float32, tag="ps3")
                            for b in range(KB):
                                nc.tensor.matmul(
                                    psum[:, :],
                                    wsb[:, b],
                                    pair(h2, b, n),
                                    start=(b == 0),
                                    stop=(b == KB - 1),
                                    perf_mode=DR,
                                )
                            t_h3 = h3pool.tile([128, 512], dt.float16,
                                               tag="h3")
                            nc.scalar.activation(t_h3[:], psum[:, :],
                                                 AF.Identity,
                                                 bias=b3_sb[:, m:m + 1])
                            nc.vector.tensor_scalar(
                                t_h3[:], t_h3[:], 1.0, -1.0,
                                mybir.AluOpType.min, mybir.AluOpType.max)
                            h3_tiles[m] = t_h3
                            # fc4 batched every 8 m-tiles, one group behind
                            # so the PE never waits on ACT/DVE
                            if m % 8 == 7 and m >= 15:
                                for mm in range(m - 15, m - 7):
                                    fc4_mm(mm)
                            # half-0 softmax tail hides under half-1 fc3
                            if n == 1:
                                if m == 2:
                                    tail_head(0)
                                elif 4 <= m <= 7:
                                    tail_j(0, m - 4)
                        for mm in range(MT - 8, MT):
                            fc4_mm(mm)

                    # ------------- final softmax tail (half 1) -------------
                    tail_head(1)
                    for j in range(NJ2):
                        tail_j(1, j)
                    sums = smp.tile([128, NH * NJ2], dt.float32, tag="sums")
                    nc.vector.tensor_reduce(sums[:], zex[:, :, :],
                                            mybir.AxisListType.X,
                                            mybir.AluOpType.add)
                    lns = smp.tile([128, NH * NJ2], dt.float32, tag="lns")
                    nc.scalar.activation(lns[:], sums[:], AF.Ln)
                    for t in range(NH * NJ2):
                        res = smp.tile([128, DOUT], dt.float32, tag=f"res{t}",
                                       name=f"res{t}")
                        nc.vector.tensor_scalar(res[:], zlog[:, t, :],
                                                lns[:, t:t + 1], None,
                                                mybir.AluOpType.subtract)
                        nc.sync.dma_start(
                            out=out[t * 128:(t + 1) * 128, :], in_=res[:])

    nc.compile()
    return nc


def _pack_inputs(x, w1, b1, w2, b2, w3, b3, w4, b4):
    """Host-side packing into the device layouts. Shared tensors are packed
    once; only xt16/xt8 differ per core."""
    f32 = np.float32
    f16 = np.float16
    x = np.asarray(x, f32).reshape(B, DIN)

    s1 = np.sign(np.asarray(w1, f32))                       # [DH, DIN]
    # fp16 stationary stack: 784 hi rows + 112 lo rows (features 672..783)
    s16 = np.zeros((K16 * 128, DH), f16)
    s16[:DIN] = s1.T
    s16[DIN:DIN + 112] = s1.T[NLO:DIN]
    w1t16 = np.ascontiguousarray(
        s16.reshape(K16, 128, MQ, MPQ * 128).transpose(2, 1, 0, 3))
    # fp8 stationary: +-2^-9 for lo features 0..671, zero-padded to 768
    s8 = np.zeros((KD * 256, DH), f32)
    s8[:NLO] = s1.T[:NLO] / LSC
    w1t8 = np.ascontiguousarray(
        s8.reshape(KD, 2, 128, MQ, MPQ * 128).transpose(3, 2, 0, 1, 4)
    ).astype(FP8)

    def pack_dr(w):
        # sign(w).T -> [mo, p, b, i, m'] DoubleRow stationary layout
        st = np.sign(np.asarray(w, f32)).T                  # [in, out]
        r = st.reshape(KB, 2, 128, MT, 128)                 # [b, i, p, mo, m']
        return np.ascontiguousarray(r.transpose(3, 2, 0, 1, 4)).astype(FP8)

    def pack_dr_swi(w):
        # [mo, p, b, 2*(127-m')+i] (A/B interleaved, reversed columns)
        st = np.sign(np.asarray(w, f32)).T                  # [in, out]
        r = st.reshape(KB, 2, 128, MT, 128)                 # [b, i, p, mo, m']
        t5 = r.transpose(3, 2, 0, 1, 4)                     # [mo, p, b, i, m']
        return np.ascontiguousarray(
            t5[:, :, :, :, ::-1].transpose(0, 1, 2, 4, 3)
            .reshape(MT, 128, KB, 256)).astype(FP8)

    w2p = pack_dr_swi(w2)
    w3p = pack_dr(w3)

    # fc4 weights: w4.T in fp16, layout [p, j, c]
    w4t = np.asarray(w4, f32).T.astype(f16)                 # [DH, DOUT]
    w4p = np.ascontiguousarray(w4t.reshape(MT, 128, DOUT).transpose(1, 0, 2))

    def pack_b(b):
        return np.ascontiguousarray(np.asarray(b, f32).reshape(MT, 128).T)

    b1p, b2p, b3p = pack_b(b1), pack_b(b2), pack_b(b3)
    b4p = np.asarray(b4, f32).reshape(DOUT, 1)

    shared = {"w1t16": w1t16, "w1t8": w1t8, "w2p": w2p, "w3p": w3p,
              "w4p": w4p, "b1p": b1p, "b2p": b2p, "b3p": b3p, "b4p": b4p,
              "identp": np.eye(DOUT, dtype=f32)}

    in_maps = []
    for c in range(CORES):
        xc = x[c * BC:(c + 1) * BC]                         # [BC, DIN]
        hi = xc.astype(f16)
        lo = (xc.astype(np.float64) - hi.astype(np.float64))
        a16 = np.zeros((K16 * 128, BC), f16)
        a16[:DIN] = hi.T
        a16[DIN:DIN + 112] = lo.T[NLO:DIN].astype(f16)
        a8 = np.zeros((KD * 256, BC), np.float64)
        a8[:NLO] = lo.T[:NLO] * LSC
        xt16c = np.ascontiguousarray(
            a16.reshape(K16, 128, BC).transpose(1, 0, 2))
        xt8c = np.ascontiguousarray(
            a8.reshape(KD, 2, 128, BC).transpose(2, 0, 1, 3)).astype(FP8)
        in_maps.append({"xt16": xt16c, "xt8": xt8c, **shared})
    return in_maps


_cached_nc = None


def kernel(x, w1, b1, w2, b2, w3, b3, w4, b4):
    global _cached_nc, last_exec_time_ns
    import os
    trace = bool(int(os.environ.get("KERNEL_TRACE", "0")))
    if _cached_nc is None:
        _cached_nc = _build_program()
    in_maps = _pack_inputs(x, w1, b1, w2, b2, w3, b3, w4, b4)
    res = run_bass_kernel_spmd(_cached_nc, in_maps, list(range(CORES)),
                               trace=trace)
    last_exec_time_ns = res.exec_time_ns
    return np.concatenate([res.results[c]["out"] for c in range(CORES)], axis=0)


# revision 16
# speedup vs baseline: 1.0017x; 1.0017x over previous
"""Binarized 4-layer MLP (8192x784 -> 6144 -> 6144 -> 6144 -> 10, log_softmax)
on 8 Trainium2 NeuronCores, data-parallel over the batch.

Per-core dataflow (batch slice of 1024, feature-major activations [feat, batch]):
  fc1: x @ sign(w1).T as a hybrid split of x: hi = fp16(x) plus 112 exact fp16
       lo rows in the 7th k-tile's padding, and the remaining 672 lo rows
       (lo = x - hi) scaled by 2^9 in fp8e4 via 3 DoubleRow blocks whose
       stationary weights are +-2^-9 (exactly representable; PE handles fp8
       subnormal weights losslessly - HW verified). All terms accumulate into
       one PSUM group, so fc1 costs 10 passes instead of 13 at ~2^-15
       relative x error, which flips only ~1e-4 of h1 signs.
  fc2/fc3: sign(h) @ sign(w).T in fp8e4 with DoubleRow perf mode (fc2 uses
       the SwInterleave stationary layout; measured identical to DoubleRow).
       All products are +-1 and partial sums are small integers, so fp32 PSUM
       accumulation is bit-exact regardless of order.
  fc4: fused into the fc3 m-loop, single fp16 pass (w4 and h3 in fp16).
  log_softmax: PE-transpose of the logits to [batch, 10] tiles, exp/sum/ln
       without max-subtraction (logits are O(1), no overflow risk).

Schedule notes:
  - startup DMAs are split small and spread over the sync/gpsimd (+scalar
    early) DGE queues in consumption order
  - h1/h2 are split into lo/hi tiles so the next layer's first matmul
    doesn't wait on the last sign() of the previous layer
  - fc3/fc4 run per batch-half; the softmax tail of half 0 hides under the
    fc3 matmuls of half 1; one Ln at the very end serves both halves
"""

import numpy as np
import ml_dtypes

import concourse.bass as bass
import concourse.mybir as mybir
from concourse import bacc
from concourse.tile import TileContext
from concourse.bass_utils import run_bass_kernel_spmd
from concourse.masks import make_identity

dt = mybir.dt

CORES = 8
B = 8192
BC = B // CORES          # 1024 batch rows per core
DIN = 784
K16 = 7                  # fc1 fp16 k-tiles (784 hi + 112 exact lo rows)
KD = 3                   # fc1 fp8 DoubleRow blocks (672 lo rows + 96 pad)
NLO = 672                # lo rows carried in fp8
LSC = 512.0              # lo scale 2^9 (weights +-2^-9)
DH = 6144
MT = DH // 128           # 48 feature tiles
MH = MT // 2             # 24 tiles per lo/hi activation buffer
KB = DH // 256           # 24 DoubleRow contraction blocks
DOUT = 10
NH = BC // 512           # 2 moving halves of 512
NJ2 = 512 // 128         # 4 output j-tiles per half
MQ = 12                  # fc1 m-groups (w1 streamed per 4 m-tiles)
MPQ = MT // MQ

BF16 = ml_dtypes.bfloat16
FP8 = mybir.dt.np(dt.float8e4)

last_exec_time_ns = None


def _build_program():
    nc = bacc.Bacc("TRN2", target_bir_lowering=False, debug=False,
                   num_devices=CORES)

    xt16 = nc.dram_tensor("xt16", [128, K16, BC], dt.float16,
                          kind="ExternalInput").ap()
    xt8 = nc.dram_tensor("xt8", [128, KD, 2, BC], dt.float8e4,
                         kind="ExternalInput").ap()
    w1t16 = nc.dram_tensor("w1t16", [MQ, 128, K16, MPQ * 128], dt.float16,
                           kind="ExternalInput").ap()
    w1t8 = nc.dram_tensor("w1t8", [MQ, 128, KD, 2, MPQ * 128], dt.float8e4,
                          kind="ExternalInput").ap()
    w2p = nc.dram_tensor("w2p", [MT, 128, KB, 256], dt.float8e4,
                         kind="ExternalInput").ap()
    w3p = nc.dram_tensor("w3p", [MT, 128, KB, 2, 128], dt.float8e4,
                         kind="ExternalInput").ap()
    w4p = nc.dram_tensor("w4p", [128, MT, DOUT], dt.float16,
                         kind="ExternalInput").ap()
    b1p = nc.dram_tensor("b1p", [128, MT], dt.float32, kind="ExternalInput").ap()
    b2p = nc.dram_tensor("b2p", [128, MT], dt.float32, kind="ExternalInput").ap()
    b3p = nc.dram_tensor("b3p", [128, MT], dt.float32, kind="ExternalInput").ap()
    b4p = nc.dram_tensor("b4p", [DOUT, 1], dt.float32, kind="ExternalInput").ap()
    out = nc.dram_tensor("out", [BC, DOUT], dt.float32, kind="ExternalOutput").ap()

    DR = mybir.MatmulPerfMode.DoubleRow
    DRSW = mybir.MatmulPerfMode.DoubleRowSwInterleave
    AF = mybir.ActivationFunctionType

    with TileContext(nc) as tc:
        with tc.tile_pool(name="consts", bufs=1) as cpool, \
             tc.tile_pool(name="h1p", bufs=1) as h1pool:
            h1 = [h1pool.tile([128, MH, BC], dt.float8e4, tag=f"h1{i}",
                              name=f"h1{i}") for i in range(2)]

            # prefetched first w2/w3 m-tiles (their zone opens mid-program)
            w2f = cpool.tile([128, KB, 256], dt.float8e4)
            w3f = cpool.tile([128, KB, 2, 128], dt.float8e4)
            b1_sb = cpool.tile([128, MT], dt.float32)
            b2_sb = cpool.tile([128, MT], dt.float32)
            b3_sb = cpool.tile([128, MT], dt.float32)
            b4_sb = cpool.tile([DOUT, 1], dt.float32)
            w4_sb = cpool.tile([128, MT, DOUT], dt.float16)
            ident = cpool.tile([DOUT, DOUT], dt.float32)

            # ---------------- fc1 ----------------
            with tc.tile_pool(name="fc1in", bufs=1) as fpool, \
                 tc.tile_pool(name="w1pool", bufs=3) as w1pool, \
                 tc.tile_pool(name="ps1", bufs=3, space="PSUM") as ps1:
                # startup DMAs, small pieces in consumption order; scalar's
                # DGE only helps before the ACT engine starts sign() work
                jobs = []
                xt_half = {}
                w1q0a = {}
                w1q0b = {}
                tiles = {}
                for k in range(K16):
                    txa = fpool.tile([128, 512], dt.float16, tag=f"xta_{k}",
                                     name=f"xta_{k}")
                    txb = fpool.tile([128, 512], dt.float16, tag=f"xtb_{k}",
                                     name=f"xtb_{k}")
                    twa = fpool.tile([128, 128], dt.float16, tag=f"w1a_{k}",
                                     name=f"w1a_{k}")
                    twb = fpool.tile([128, 384], dt.float16, tag=f"w1b_{k}",
                                     name=f"w1b_{k}")
                    tiles[k] = (txa, txb, twa, twb)
                    xt_half[(k, 0)] = txa[:, :]
                    xt_half[(k, 1)] = txb[:, :]
                    w1q0a[k] = twa
                    w1q0b[k] = twb
                x8t = fpool.tile([128, KD, 2, BC], dt.float8e4)
                w1q08 = fpool.tile([128, KD, 2, MPQ * 128], dt.float8e4)

                def kjobs(k):
                    txa, txb, twa, twb = tiles[k]
                    return [(twa[:, :], w1t16[0, :, k, 0:128]),
                            (txa[:, :], xt16[:, k, 0:512]),
                            (txb[:, :], xt16[:, k, 512:1024]),
                            (twb[:, :], w1t16[0, :, k, 128:512])]

                # consumption order: k=0, then the sandwiched DR operands,
                # then k=1..6, then the m>0 parts of the q0 weights
                jobs += kjobs(0)
                jobs += [(w1q08[:, :, :, 0:128], w1t8[0, :, :, :, 0:128])]
                for b in range(KD):
                    jobs += [(x8t[:, b], xt8[:, b])]
                for k in range(1, K16):
                    jobs += kjobs(k)
                jobs += [(w1q08[:, :, :, 128:512], w1t8[0, :, :, :, 128:512])]
                for i, (dst, src) in enumerate(jobs):
                    q = ([nc.sync, nc.gpsimd, nc.scalar][i % 3] if i < 12
                         else [nc.sync, nc.gpsimd][i % 2])
                    q.dma_start(out=dst, in_=src)

                nc.sync.dma_start(out=b1_sb[:], in_=b1p[:])
                nc.gpsimd.dma_start(out=b2_sb[:], in_=b2p[:])
                nc.sync.dma_start(out=b3_sb[:], in_=b3p[:])
                nc.gpsimd.dma_start(out=b4_sb[:], in_=b4p[:])
                nc.sync.dma_start(out=w4_sb[:], in_=w4p[:])
                make_identity(nc, ident[:])

                for q in range(MQ):
                    if q == 0:
                        def lhs16(k, mi):
                            if mi == 0:
                                return w1q0a[k][:, :]
                            return w1q0b[k][:, (mi - 1) * 128:mi * 128]

                        def lhs8(b, mi):
                            return w1q08[:, b, :, mi * 128:(mi + 1) * 128]
                    else:
                        # q==1 must load during the startup crunch (sync);
                        # later groups dispatch from the scalar queue, which
                        # is backed up behind sign() ACTs — a free just-in-
                        # time delay that keeps these 1.3MB streams out of
                        # the startup DMA crunch
                        wq = nc.sync if q == 1 else nc.scalar
                        w1q16 = w1pool.tile([128, K16, MPQ * 128], dt.float16,
                                            tag="w1q16")
                        wq.dma_start(out=w1q16[:], in_=w1t16[q])
                        w1q8 = w1pool.tile([128, KD, 2, MPQ * 128],
                                           dt.float8e4, tag="w1q8")
                        (nc.gpsimd if q == 1 else nc.scalar).dma_start(
                            out=w1q8[:], in_=w1t8[q])

                        def lhs16(k, mi, w1q16=w1q16):
                            return w1q16[:, k, mi * 128:(mi + 1) * 128]

                        def lhs8(b, mi, w1q8=w1q8):
                            return w1q8[:, b, :, mi * 128:(mi + 1) * 128]
                    for mi in range(MPQ):
                        m = q * MPQ + mi
                        psum = ps1.tile([128, BC], dt.float32, tag="ps1")
                        # DR blocks sandwiched mid-group: a DR matmul at an
                        # accumulation-group boundary costs an extra ~200ns
                        # (unpipelined LDWEIGHTS); fp16 edges don't
                        for n in range(NH):
                            nc.tensor.matmul(
                                psum[:, n * 512:(n + 1) * 512],
                                lhs16(0, mi),
                                xt_half[(0, n)],
                                start=True,
                                stop=False,
                            )
                        for b in range(KD):
                            for n in range(NH):
                                nc.tensor.matmul(
                                    psum[:, n * 512:(n + 1) * 512],
                                    lhs8(b, mi),
                                    x8t[:, b, :, n * 512:(n + 1) * 512],
                                    start=False,
                                    stop=False,
                                    perf_mode=DR,
                                )
                        for k in range(1, K16):
                            for n in range(NH):
                                nc.tensor.matmul(
                                    psum[:, n * 512:(n + 1) * 512],
                                    lhs16(k, mi),
                                    xt_half[(k, n)],
                                    start=False,
                                    stop=(k == K16 - 1),
                                )
                        nc.scalar.sign(h1[m // MH][:, m % MH, :], psum[:, :],
                                       bias=b1_sb[:, m:m + 1])
                        if m == 20:
                            # deferred w2/w3 first-tile prefetch: scalar's
                            # queue is behind ~20 signs, so these 786KB
                            # loads dispatch ~100us in, clear of the crunch
                            nc.scalar.dma_start(out=w2f[:], in_=w2p[0])
                        elif m == 21:
                            nc.scalar.dma_start(out=w3f[:], in_=w3p[0])

            # ---------------- fc2 ----------------
            def pair(h, b, n):
                # moving [128, 2, 512] for DR block b out of lo/hi buffers
                t = 2 * b
                return h[t // MH][:, t % MH:t % MH + 2, n * 512:(n + 1) * 512]

            with tc.tile_pool(name="h2p", bufs=1) as h2pool:
                h2 = [h2pool.tile([128, MH, BC], dt.float8e4, tag=f"h2{i}",
                                  name=f"h2{i}") for i in range(2)]
                with tc.tile_pool(name="w2pool", bufs=3) as w2pool, \
                     tc.tile_pool(name="ps2", bufs=3, space="PSUM") as ps2:
                    for m in range(MT):
                        if m == 0:
                            wsb = w2f
                        else:
                            wsb = w2pool.tile([128, KB, 256], dt.float8e4,
                                              tag="w2")
                            nc.sync.dma_start(out=wsb[:], in_=w2p[m])
                        psum = ps2.tile([128, BC], dt.float32, tag="ps2")
                        for n in range(NH):
                            for b in range(KB):
                                nc.tensor.matmul(
                                    psum[:, n * 512:(n + 1) * 512],
                                    wsb[:, b],
                                    pair(h1, b, n),
                                    start=(b == 0),
                                    stop=(b == KB - 1),
                                    perf_mode=DRSW,
                                )
                        nc.scalar.sign(h2[m // MH][:, m % MH, :], psum[:, :],
                                       bias=b2_sb[:, m:m + 1])

                # ------------- fc3 + fused fc4 + log_softmax -------------
                with tc.tile_pool(name="lgp", bufs=1, space="PSUM") as lgp, \
                     tc.tile_pool(name="tpp", bufs=2, space="PSUM") as tpp, \
                     tc.tile_pool(name="smp", bufs=1) as smp, \
                     tc.tile_pool(name="w3pool", bufs=3) as w3pool, \
                     tc.tile_pool(name="h3pool", bufs=18) as h3pool, \
                     tc.tile_pool(name="ps3", bufs=3, space="PSUM") as ps3:
                    zex = smp.tile([128, NH * NJ2, DOUT], dt.float32)
                    zlog = smp.tile([128, NH * NJ2, DOUT], dt.float32)
                    lg_psums = {}
                    lg_sbs = {}

                    def tail_head(n):
                        lg_sb = smp.tile([DOUT, 512], dt.float32,
                                         tag=f"lgsb{n}", name=f"lgsb{n}")
                        nc.scalar.activation(lg_sb[:], lg_psums[n][:],
                                             AF.Identity, bias=b4_sb[:, 0:1])
                        lg_sbs[n] = lg_sb

                    def tail_j(n, j):
                        t = n * NJ2 + j
                        tp = tpp.tile([128, DOUT], dt.float32, tag="tp",
                                      name="tp")
                        nc.tensor.transpose(
                            tp[:], lg_sbs[n][:, j * 128:(j + 1) * 128],
                            ident[:])
                        nc.scalar.activation(zex[:, t, :], tp[:], AF.Exp)
                        nc.vector.tensor_scalar_add(zlog[:, t, :], tp[:], 0.0)

                    for n in range(NH):
                        lg_psum = lgp.tile([DOUT, 512], dt.float32,
                                           tag=f"lg{n}", name=f"lg{n}")
                        lg_psums[n] = lg_psum
                        h3_tiles = [None] * MT

                        def fc4_mm(m, lg_psum=lg_psum, h3_tiles=h3_tiles):
                            nc.tensor.matmul(
                                lg_psum[:, :],
                                w4_sb[:, m, :],
                                h3_tiles[m][:, :],
                                start=(m == 0),
                                stop=(m == MT - 1),
                            )

                        for m in range(MT):
                            if n == 0 and m == 0:
                                wsb = w3f
                            else:
                                wsb = w3pool.tile([128, KB, 2, 128],
                                                  dt.float8e4, tag="w3")
                                (nc.sync if m % 2 else nc.gpsimd).dma_start(
                                    out=wsb[:], in_=w3p[m])
                            psum = ps3.tile([128, 512], dt.float32, tag="ps3")
                            for b in range(KB):
                                nc.tensor.matmul(
                                    psum[:, :],
                                    wsb[:, b],
                                    pair(h2, b, n),
                                    start=(b == 0),
                                    stop=(b == KB - 1),
                                    perf_mode=DR,
                                )
                            t_h3 = h3pool.tile([128, 512], dt.float16,
                                               tag="h3")
                            nc.scalar.activation(t_h3[:], psum[:, :],
                                                 AF.Identity,
                                                 bias=b3_sb[:, m:m + 1])
                            nc.vector.tensor_scalar(
                                t_h3[:], t_h3[:], 1.0, -1.0,
                                mybir.AluOpType.min, mybir.AluOpType.max)
                            h3_tiles[m] = t_h3
                            # fc4 batched every 8 m-tiles, one group behind
                            # so the PE never waits on ACT/DVE
                            if m % 8 == 7 and m >= 15:
                                for mm in range(m - 15, m - 7):
                                    fc4_mm(mm)
                            # half-0 softmax tail hides under half-1 fc3
                            if n == 1:
                                if m == 2:
                                    tail_head(0)
                                elif 4 <= m <= 7:
                                    tail_j(0, m - 4)
                        for mm in range(MT - 8, MT):
                            fc4_mm(mm)

                    # ------------- final softmax tail (half 1) -------------
                    tail_head(1)
                    for j in range(NJ2):
                        tail_j(1, j)
                    sums = smp.tile([128, NH * NJ2], dt.float32, tag="sums")
                    nc.vector.tensor_reduce(sums[:], zex[:, :, :],
                                            mybir.AxisListType.X,
                                            mybir.AluOpType.add)
                    lns = smp.tile([128, NH * NJ2], dt.float32, tag="lns")
                    nc.scalar.activation(lns[:], sums[:], AF.Ln)
                    for t in range(NH * NJ2):
                        res = smp.tile([128, DOUT], dt.float32, tag=f"res{t}",
                                       name=f"res{t}")
                        nc.vector.tensor_scalar(res[:], zlog[:, t, :],
                                                lns[:, t:t + 1], None,
                                                mybir.AluOpType.subtract)
                        nc.sync.dma_start(
                            out=out[t * 128:(t + 1) * 128, :], in_=res[:])

    nc.compile()
    return nc


def _pack_inputs(x, w1, b1, w2, b2, w3, b3, w4, b4):
    """Host-side packing into the device layouts. Shared tensors are packed
    once; only xt16/xt8 differ per core."""
    f32 = np.float32
    f16 = np.float16
    x = np.asarray(x, f32).reshape(B, DIN)

    s1 = np.sign(np.asarray(w1, f32))                       # [DH, DIN]
    # fp16 stationary stack: 784 hi rows + 112 lo rows (features 672..783)
    s16 = np.zeros((K16 * 128, DH), f16)
    s16[:DIN] = s1.T
    s16[DIN:DIN + 112] = s1.T[NLO:DIN]
    w1t16 = np.ascontiguousarray(
        s16.reshape(K16, 128, MQ, MPQ * 128).transpose(2, 1, 0, 3))
    # fp8 stationary: +-2^-9 for lo features 0..671, zero-padded to 768
    s8 = np.zeros((KD * 256, DH), f32)
    s8[:NLO] = s1.T[:NLO] / LSC
    w1t8 = np.ascontiguousarray(
        s8.reshape(KD, 2, 128, MQ, MPQ * 128).transpose(3, 2, 0, 1, 4)
    ).astype(FP8)

    def pack_dr(w):
        # sign(w).T -> [mo, p, b, i, m'] DoubleRow stationary layout
        st = np.sign(np.asarray(w, f32)).T                  # [in, out]
        r = st.reshape(KB, 2, 128, MT, 128)                 # [b, i, p, mo, m']
        return np.ascontiguousarray(r.transpose(3, 2, 0, 1, 4)).astype(FP8)

    def pack_dr_swi(w):
        # [mo, p, b, 2*(127-m')+i] (A/B interleaved, reversed columns)
        st = np.sign(np.asarray(w, f32)).T                  # [in, out]
        r = st.reshape(KB, 2, 128, MT, 128)                 # [b, i, p, mo, m']
        t5 = r.transpose(3, 2, 0, 1, 4)                     # [mo, p, b, i, m']
        return np.ascontiguousarray(
            t5[:, :, :, :, ::-1].transpose(0, 1, 2, 4, 3)
            .reshape(MT, 128, KB, 256)).astype(FP8)

    w2p = pack_dr_swi(w2)
    w3p = pack_dr(w3)

    # fc4 weights: w4.T in fp16, layout [p, j, c]
    w4t = np.asarray(w4, f32).T.astype(f16)                 # [DH, DOUT]
    w4p = np.ascontiguousarray(w4t.reshape(MT, 128, DOUT).transpose(1, 0, 2))

    def pack_b(b):
        return np.ascontiguousarray(np.asarray(b, f32).reshape(MT, 128).T)

    b1p, b2p, b3p = pack_b(b1), pack_b(b2), pack_b(b3)
    b4p = np.asarray(b4, f32).reshape(DOUT, 1)

    shared = {"w1t16": w1t16, "w1t8": w1t8, "w2p": w2p, "w3p": w3p,
              "w4p": w4p, "b1p": b1p, "b2p": b2p, "b3p": b3p, "b4p": b4p}

    in_maps = []
    for c in range(CORES):
        xc = x[c * BC:(c + 1) * BC]                         # [BC, DIN]
        hi = xc.astype(f16)
        lo = (xc.astype(np.float64) - hi.astype(np.float64))
        a16 = np.zeros((K16 * 128, BC), f16)
        a16[:DIN] = hi.T
        a16[DIN:DIN + 112] = lo.T[NLO:DIN].astype(f16)
        a8 = np.zeros((KD * 256, BC), np.float64)
        a8[:NLO] = lo.T[:NLO] * LSC
        xt16c = np.ascontiguousarray(
            a16.reshape(K16, 128, BC).transpose(1, 0, 2))
        xt8c = np.ascontiguousarray(
            a8.reshape(KD, 2, 128, BC).transpose(2, 0, 1, 3)).astype(FP8)
        in_maps.append({"xt16": xt16c, "xt8": xt8c, **shared})
    return in_maps


_cached_nc = None


def kernel(x, w1, b1, w2, b2, w3, b3, w4, b4):
    global _cached_nc, last_exec_time_ns
    import os
    trace = bool(int(os.environ.get("KERNEL_TRACE", "0")))
    if _cached_nc is None:
        _cached_nc = _build_program()
    in_maps = _pack_inputs(x, w1, b1, w2, b2, w3, b3, w4, b4)
    res = run_bass_kernel_spmd(_cached_nc, in_maps, list(range(CORES)),
                               trace=trace)
    last_exec_time_ns = res.exec_time_ns
    return np.concatenate([res.results[c]["out"] for c in range(CORES)], axis=0)


# revision 17
# speedup vs baseline: 1.0081x; 1.0063x over previous
"""Binarized 4-layer MLP (8192x784 -> 6144 -> 6144 -> 6144 -> 10, log_softmax)
on 8 Trainium2 NeuronCores, data-parallel over the batch.

Per-core dataflow (batch slice of 1024, feature-major activations [feat, batch]):
  fc1: x @ sign(w1).T as a hybrid split of x: hi = fp16(x) plus 112 exact fp16
       lo rows in the 7th k-tile's padding, and the remaining 672 lo rows
       (lo = x - hi) scaled by 2^9 in fp8e4 via 3 DoubleRow blocks whose
       stationary weights are +-2^-9 (exactly representable; PE handles fp8
       subnormal weights losslessly - HW verified). All terms accumulate into
       one PSUM group, so fc1 costs 10 passes instead of 13 at ~2^-15
       relative x error, which flips only ~1e-4 of h1 signs.
  fc2/fc3: sign(h) @ sign(w).T in fp8e4 with DoubleRow perf mode (fc2 uses
       the SwInterleave stationary layout; measured identical to DoubleRow).
       All products are +-1 and partial sums are small integers, so fp32 PSUM
       accumulation is bit-exact regardless of order.
  fc4: fused into the fc3 m-loop, single fp16 pass (w4 and h3 in fp16).
  log_softmax: PE-transpose of the logits to [batch, 10] tiles, exp/sum/ln
       without max-subtraction (logits are O(1), no overflow risk).

Schedule notes:
  - startup DMAs are split small and spread over the sync/gpsimd (+scalar
    early) DGE queues in consumption order
  - h1/h2 are split into lo/hi tiles so the next layer's first matmul
    doesn't wait on the last sign() of the previous layer
  - fc3/fc4 run per batch-half; the softmax tail of half 0 hides under the
    fc3 matmuls of half 1; one Ln at the very end serves both halves
"""

import numpy as np
import ml_dtypes

import concourse.bass as bass
import concourse.mybir as mybir
from concourse import bacc
from concourse.tile import TileContext
from concourse.bass_utils import run_bass_kernel_spmd
from concourse.masks import make_identity

dt = mybir.dt

CORES = 8
B = 8192
BC = B // CORES          # 1024 batch rows per core
DIN = 784
K16 = 7                  # fc1 fp16 k-tiles (784 hi + 112 exact lo rows)
KD = 3                   # fc1 fp8 DoubleRow blocks (672 lo rows + 96 pad)
NLO = 672                # lo rows carried in fp8
LSC = 512.0              # lo scale 2^9 (weights +-2^-9)
DH = 6144
MT = DH // 128           # 48 feature tiles
MH = MT // 2             # 24 tiles per lo/hi activation buffer
KB = DH // 256           # 24 DoubleRow contraction blocks
DOUT = 10
NH = BC // 512           # 2 moving halves of 512
NJ2 = 512 // 128         # 4 output j-tiles per half
MQ = 12                  # fc1 m-groups (w1 streamed per 4 m-tiles)
MPQ = MT // MQ

BF16 = ml_dtypes.bfloat16
FP8 = mybir.dt.np(dt.float8e4)

last_exec_time_ns = None


def _build_program():
    nc = bacc.Bacc("TRN2", target_bir_lowering=False, debug=False,
                   num_devices=CORES)

    xt16 = nc.dram_tensor("xt16", [128, K16, BC], dt.float16,
                          kind="ExternalInput").ap()
    xt8 = nc.dram_tensor("xt8", [128, KD, 2, BC], dt.float8e4,
                         kind="ExternalInput").ap()
    w1t16 = nc.dram_tensor("w1t16", [MQ, 128, K16, MPQ * 128], dt.float16,
                           kind="ExternalInput").ap()
    w1t8 = nc.dram_tensor("w1t8", [MQ, 128, KD, 2, MPQ * 128], dt.float8e4,
                          kind="ExternalInput").ap()
    w2p = nc.dram_tensor("w2p", [MT, 128, KB, 256], dt.float8e4,
                         kind="ExternalInput").ap()
    w3p = nc.dram_tensor("w3p", [MT, 128, KB, 2, 128], dt.float8e4,
                         kind="ExternalInput").ap()
    w4p = nc.dram_tensor("w4p", [128, MT, DOUT], dt.float16,
                         kind="ExternalInput").ap()
    b1p = nc.dram_tensor("b1p", [128, MT], dt.float32, kind="ExternalInput").ap()
    b2p = nc.dram_tensor("b2p", [128, MT], dt.float32, kind="ExternalInput").ap()
    b3p = nc.dram_tensor("b3p", [128, MT], dt.float32, kind="ExternalInput").ap()
    b4p = nc.dram_tensor("b4p", [DOUT, 1], dt.float32, kind="ExternalInput").ap()
    selp = nc.dram_tensor("selp", [128, DOUT], dt.float32,
                          kind="ExternalInput").ap()
    out = nc.dram_tensor("out", [BC, DOUT], dt.float32, kind="ExternalOutput").ap()

    DR = mybir.MatmulPerfMode.DoubleRow
    DRSW = mybir.MatmulPerfMode.DoubleRowSwInterleave
    AF = mybir.ActivationFunctionType

    with TileContext(nc) as tc:
        with tc.tile_pool(name="consts", bufs=1) as cpool, \
             tc.tile_pool(name="h1p", bufs=1) as h1pool:
            h1 = [h1pool.tile([128, MH, BC], dt.float8e4, tag=f"h1{i}",
                              name=f"h1{i}") for i in range(2)]

            # prefetched first w2/w3 m-tiles (their zone opens mid-program)
            w2f = cpool.tile([128, KB, 256], dt.float8e4)
            w3f = cpool.tile([128, KB, 2, 128], dt.float8e4)
            b1_sb = cpool.tile([128, MT], dt.float32)
            b2_sb = cpool.tile([128, MT], dt.float32)
            b3_sb = cpool.tile([128, MT], dt.float32)
            b4_sb = cpool.tile([DOUT, 1], dt.float32)
            w4_sb = cpool.tile([128, MT, DOUT], dt.float16)
            ident = cpool.tile([DOUT, DOUT], dt.float32)
            sel_sb = cpool.tile([128, DOUT], dt.float32)

            # ---------------- fc1 ----------------
            with tc.tile_pool(name="fc1in", bufs=1) as fpool, \
                 tc.tile_pool(name="w1pool", bufs=3) as w1pool, \
                 tc.tile_pool(name="ps1", bufs=3, space="PSUM") as ps1:
                # startup DMAs, small pieces in consumption order; scalar's
                # DGE only helps before the ACT engine starts sign() work
                jobs = []
                xt_half = {}
                w1q0a = {}
                w1q0b = {}
                tiles = {}
                for k in range(K16):
                    txa = fpool.tile([128, 512], dt.float16, tag=f"xta_{k}",
                                     name=f"xta_{k}")
                    txb = fpool.tile([128, 512], dt.float16, tag=f"xtb_{k}",
                                     name=f"xtb_{k}")
                    twa = fpool.tile([128, 128], dt.float16, tag=f"w1a_{k}",
                                     name=f"w1a_{k}")
                    twb = fpool.tile([128, 384], dt.float16, tag=f"w1b_{k}",
                                     name=f"w1b_{k}")
                    tiles[k] = (txa, txb, twa, twb)
                    xt_half[(k, 0)] = txa[:, :]
                    xt_half[(k, 1)] = txb[:, :]
                    w1q0a[k] = twa
                    w1q0b[k] = twb
                x8t = fpool.tile([128, KD, 2, BC], dt.float8e4)
                w1q08 = fpool.tile([128, KD, 2, MPQ * 128], dt.float8e4)

                def kjobs(k):
                    txa, txb, twa, twb = tiles[k]
                    return [(twa[:, :], w1t16[0, :, k, 0:128]),
                            (txa[:, :], xt16[:, k, 0:512]),
                            (txb[:, :], xt16[:, k, 512:1024]),
                            (twb[:, :], w1t16[0, :, k, 128:512])]

                # consumption order: k=0, then the sandwiched DR operands,
                # then k=1..6, then the m>0 parts of the q0 weights
                jobs += kjobs(0)
                jobs += [(w1q08[:, :, :, 0:128], w1t8[0, :, :, :, 0:128])]
                for b in range(KD):
                    jobs += [(x8t[:, b], xt8[:, b])]
                for k in range(1, K16):
                    jobs += kjobs(k)
                jobs += [(w1q08[:, :, :, 128:512], w1t8[0, :, :, :, 128:512])]
                for i, (dst, src) in enumerate(jobs):
                    q = ([nc.sync, nc.gpsimd, nc.scalar][i % 3] if i < 12
                         else [nc.sync, nc.gpsimd][i % 2])
                    q.dma_start(out=dst, in_=src)

                nc.sync.dma_start(out=b1_sb[:], in_=b1p[:])
                nc.gpsimd.dma_start(out=b2_sb[:], in_=b2p[:])
                nc.sync.dma_start(out=b3_sb[:], in_=b3p[:])
                nc.gpsimd.dma_start(out=b4_sb[:], in_=b4p[:])
                nc.sync.dma_start(out=w4_sb[:], in_=w4p[:])
                nc.gpsimd.dma_start(out=sel_sb[:], in_=selp[:])
                make_identity(nc, ident[:])

                for q in range(MQ):
                    if q == 0:
                        def lhs16(k, mi):
                            if mi == 0:
                                return w1q0a[k][:, :]
                            return w1q0b[k][:, (mi - 1) * 128:mi * 128]

                        def lhs8(b, mi):
                            return w1q08[:, b, :, mi * 128:(mi + 1) * 128]
                    else:
                        # q==1 must load during the startup crunch (sync);
                        # later groups dispatch from the scalar queue, which
                        # is backed up behind sign() ACTs — a free just-in-
                        # time delay that keeps these 1.3MB streams out of
                        # the startup DMA crunch
                        wq = nc.sync if q == 1 else nc.scalar
                        w1q16 = w1pool.tile([128, K16, MPQ * 128], dt.float16,
                                            tag="w1q16")
                        wq.dma_start(out=w1q16[:], in_=w1t16[q])
                        w1q8 = w1pool.tile([128, KD, 2, MPQ * 128],
                                           dt.float8e4, tag="w1q8")
                        (nc.gpsimd if q == 1 else nc.scalar).dma_start(
                            out=w1q8[:], in_=w1t8[q])

                        def lhs16(k, mi, w1q16=w1q16):
                            return w1q16[:, k, mi * 128:(mi + 1) * 128]

                        def lhs8(b, mi, w1q8=w1q8):
                            return w1q8[:, b, :, mi * 128:(mi + 1) * 128]
                    for mi in range(MPQ):
                        m = q * MPQ + mi
                        psum = ps1.tile([128, BC], dt.float32, tag="ps1")
                        # DR blocks sandwiched mid-group: a DR matmul at an
                        # accumulation-group boundary costs an extra ~200ns
                        # (unpipelined LDWEIGHTS); fp16 edges don't
                        for n in range(NH):
                            nc.tensor.matmul(
                                psum[:, n * 512:(n + 1) * 512],
                                lhs16(0, mi),
                                xt_half[(0, n)],
                                start=True,
                                stop=False,
                            )
                        for b in range(KD):
                            for n in range(NH):
                                nc.tensor.matmul(
                                    psum[:, n * 512:(n + 1) * 512],
                                    lhs8(b, mi),
                                    x8t[:, b, :, n * 512:(n + 1) * 512],
                                    start=False,
                                    stop=False,
                                    perf_mode=DR,
                                )
                        for k in range(1, K16):
                            for n in range(NH):
                                nc.tensor.matmul(
                                    psum[:, n * 512:(n + 1) * 512],
                                    lhs16(k, mi),
                                    xt_half[(k, n)],
                                    start=False,
                                    stop=(k == K16 - 1),
                                )
                        nc.scalar.sign(h1[m // MH][:, m % MH, :], psum[:, :],
                                       bias=b1_sb[:, m:m + 1])
                        if m == 20:
                            # deferred w2/w3 first-tile prefetch: scalar's
                            # queue is behind ~20 signs, so these 786KB
                            # loads dispatch ~100us in, clear of the crunch
                            nc.scalar.dma_start(out=w2f[:], in_=w2p[0])
                        elif m == 21:
                            nc.scalar.dma_start(out=w3f[:], in_=w3p[0])

            # ---------------- fc2 ----------------
            def pair(h, b, n):
                # moving [128, 2, 512] for DR block b out of lo/hi buffers
                t = 2 * b
                return h[t // MH][:, t % MH:t % MH + 2, n * 512:(n + 1) * 512]

            with tc.tile_pool(name="h2p", bufs=1) as h2pool:
                h2 = [h2pool.tile([128, MH, BC], dt.float8e4, tag=f"h2{i}",
                                  name=f"h2{i}") for i in range(2)]
                with tc.tile_pool(name="w2pool", bufs=3) as w2pool, \
                     tc.tile_pool(name="ps2", bufs=3, space="PSUM") as ps2:
                    for m in range(MT):
                        if m == 0:
                            wsb = w2f
                        else:
                            wsb = w2pool.tile([128, KB, 256], dt.float8e4,
                                              tag="w2")
                            nc.sync.dma_start(out=wsb[:], in_=w2p[m])
                        psum = ps2.tile([128, BC], dt.float32, tag="ps2")
                        for n in range(NH):
                            for b in range(KB):
                                nc.tensor.matmul(
                                    psum[:, n * 512:(n + 1) * 512],
                                    wsb[:, b],
                                    pair(h1, b, n),
                                    start=(b == 0),
                                    stop=(b == KB - 1),
                                    perf_mode=DRSW,
                                )
                        nc.scalar.sign(h2[m // MH][:, m % MH, :], psum[:, :],
                                       bias=b2_sb[:, m:m + 1])

                # ------------- fc3 + fused fc4 + log_softmax -------------
                with tc.tile_pool(name="lgp", bufs=1, space="PSUM") as lgp, \
                     tc.tile_pool(name="zsp", bufs=1, space="PSUM") as zsp, \
                     tc.tile_pool(name="tpp", bufs=2, space="PSUM") as tpp, \
                     tc.tile_pool(name="smp", bufs=1) as smp, \
                     tc.tile_pool(name="w3pool", bufs=3) as w3pool, \
                     tc.tile_pool(name="h3pool", bufs=18) as h3pool, \
                     tc.tile_pool(name="ps3", bufs=3, space="PSUM") as ps3:
                    zex = smp.tile([128, NH * NJ2, DOUT], dt.float32)
                    zlog = smp.tile([128, NH * NJ2, DOUT], dt.float32)
                    lg_psums = {}
                    lg_sbs = {}

                    def tail_head(n):
                        lgcp = smp.tile([128, 512], dt.float32,
                                        tag="lgcp", name="lgcp")
                        nc.scalar.copy(lgcp[:], lg_psums[n][:])
                        zsum = zsp.tile([DOUT, 512], dt.float32, tag="zs",
                                        name="zs")
                        nc.tensor.matmul(zsum[:], sel_sb[:], lgcp[:],
                                         start=True, stop=True)
                        lg_sb = smp.tile([DOUT, 512], dt.float32,
                                         tag=f"lgsb{n}", name=f"lgsb{n}")
                        nc.scalar.activation(lg_sb[:], zsum[:],
                                             AF.Identity, bias=b4_sb[:, 0:1])
                        lg_sbs[n] = lg_sb

                    def tail_j(n, j):
                        t = n * NJ2 + j
                        tp = tpp.tile([128, DOUT], dt.float32, tag="tp",
                                      name="tp")
                        nc.tensor.transpose(
                            tp[:], lg_sbs[n][:, j * 128:(j + 1) * 128],
                            ident[:])
                        nc.scalar.activation(zex[:, t, :], tp[:], AF.Exp)
                        nc.vector.tensor_scalar_add(zlog[:, t, :], tp[:], 0.0)

                    for n in range(NH):
                        lg_psum = lgp.tile([128, 512], dt.float32,
                                           tag=f"lg{n}", name=f"lg{n}")
                        lg_psums[n] = lg_psum
                        h3_tiles = [None] * MT

                        def fc4_mm(m, lg_psum=lg_psum, h3_tiles=h3_tiles):
                            # 4 interleaved accumulators in distinct 32-col
                            # PE subarrays: quads of these run concurrently
                            g = m % 4
                            nc.tensor.matmul(
                                lg_psum[32 * g:32 * g + DOUT, :],
                                w4_sb[:, m, :],
                                h3_tiles[m][:, :],
                                start=(m < 4),
                                stop=(m >= MT - 4),
                                tile_position=(0, 32 * g),
                                skip_group_check=True,
                            )

                        for m in range(MT):
                            if n == 0 and m == 0:
                                wsb = w3f
                            else:
                                wsb = w3pool.tile([128, KB, 2, 128],
                                                  dt.float8e4, tag="w3")
                                (nc.sync if m % 2 else nc.gpsimd).dma_start(
                                    out=wsb[:], in_=w3p[m])
                            psum = ps3.tile([128, 512], dt.float32, tag="ps3")
                            for b in range(KB):
                                nc.tensor.matmul(
                                    psum[:, :],
                                    wsb[:, b],
                                    pair(h2, b, n),
                                    start=(b == 0),
                                    stop=(b == KB - 1),
                                    perf_mode=DR,
                                )
                            t_h3 = h3pool.tile([128, 512], dt.float16,
                                               tag="h3")
                            nc.scalar.activation(t_h3[:], psum[:, :],
                                                 AF.Identity,
                                                 bias=b3_sb[:, m:m + 1])
                            nc.vector.tensor_scalar(
                                t_h3[:], t_h3[:], 1.0, -1.0,
                                mybir.AluOpType.min, mybir.AluOpType.max)
                            h3_tiles[m] = t_h3
                            # fc4 batched every 8 m-tiles, one group behind
                            # so the PE never waits on ACT/DVE
                            if m % 8 == 7 and m >= 15:
                                for mm in range(m - 15, m - 7):
                                    fc4_mm(mm)
                            # half-0 softmax tail hides under half-1 fc3
                            if n == 1:
                                if m == 2:
                                    tail_head(0)
                                elif 4 <= m <= 7:
                                    tail_j(0, m - 4)
                        for mm in range(MT - 8, MT):
                            fc4_mm(mm)

                    # ------------- final softmax tail (half 1) -------------
                    tail_head(1)
                    for j in range(NJ2):
                        tail_j(1, j)
                    sums = smp.tile([128, NH * NJ2], dt.float32, tag="sums")
                    nc.vector.tensor_reduce(sums[:], zex[:, :, :],
                                            mybir.AxisListType.X,
                                            mybir.AluOpType.add)
                    lns = smp.tile([128, NH * NJ2], dt.float32, tag="lns")
                    nc.scalar.activation(lns[:], sums[:], AF.Ln)
                    for t in range(NH * NJ2):
                        res = smp.tile([128, DOUT], dt.float32, tag=f"res{t}",
                                       name=f"res{t}")
                        nc.vector.tensor_scalar(res[:], zlog[:, t, :],
                                                lns[:, t:t + 1], None,
                                                mybir.AluOpType.subtract)
                        nc.sync.dma_start(
                            out=out[t * 128:(t + 1) * 128, :], in_=res[:])

    nc.compile()
    return nc


def _pack_inputs(x, w1, b1, w2, b2, w3, b3, w4, b4):
    """Host-side packing into the device layouts. Shared tensors are packed
    once; only xt16/xt8 differ per core."""
    f32 = np.float32
    f16 = np.float16
    x = np.asarray(x, f32).reshape(B, DIN)

    s1 = np.sign(np.asarray(w1, f32))                       # [DH, DIN]
    # fp16 stationary stack: 784 hi rows + 112 lo rows (features 672..783)
    s16 = np.zeros((K16 * 128, DH), f16)
    s16[:DIN] = s1.T
    s16[DIN:DIN + 112] = s1.T[NLO:DIN]
    w1t16 = np.ascontiguousarray(
        s16.reshape(K16, 128, MQ, MPQ * 128).transpose(2, 1, 0, 3))
    # fp8 stationary: +-2^-9 for lo features 0..671, zero-padded to 768
    s8 = np.zeros((KD * 256, DH), f32)
    s8[:NLO] = s1.T[:NLO] / LSC
    w1t8 = np.ascontiguousarray(
        s8.reshape(KD, 2, 128, MQ, MPQ * 128).transpose(3, 2, 0, 1, 4)
    ).astype(FP8)

    def pack_dr(w):
        # sign(w).T -> [mo, p, b, i, m'] DoubleRow stationary layout
        st = np.sign(np.asarray(w, f32)).T                  # [in, out]
        r = st.reshape(KB, 2, 128, MT, 128)                 # [b, i, p, mo, m']
        return np.ascontiguousarray(r.transpose(3, 2, 0, 1, 4)).astype(FP8)

    def pack_dr_swi(w):
        # [mo, p, b, 2*(127-m')+i] (A/B interleaved, reversed columns)
        st = np.sign(np.asarray(w, f32)).T                  # [in, out]
        r = st.reshape(KB, 2, 128, MT, 128)                 # [b, i, p, mo, m']
        t5 = r.transpose(3, 2, 0, 1, 4)                     # [mo, p, b, i, m']
        return np.ascontiguousarray(
            t5[:, :, :, :, ::-1].transpose(0, 1, 2, 4, 3)
            .reshape(MT, 128, KB, 256)).astype(FP8)

    w2p = pack_dr_swi(w2)
    w3p = pack_dr(w3)

    # fc4 weights: w4.T in fp16, layout [p, j, c]
    w4t = np.asarray(w4, f32).T.astype(f16)                 # [DH, DOUT]
    w4p = np.ascontiguousarray(w4t.reshape(MT, 128, DOUT).transpose(1, 0, 2))

    def pack_b(b):
        return np.ascontiguousarray(np.asarray(b, f32).reshape(MT, 128).T)

    b1p, b2p, b3p = pack_b(b1), pack_b(b2), pack_b(b3)
    b4p = np.asarray(b4, f32).reshape(DOUT, 1)
    selp = np.zeros((128, DOUT), f32)
    for g in range(4):
        selp[32 * g + np.arange(DOUT), np.arange(DOUT)] = 1.0

    shared = {"w1t16": w1t16, "w1t8": w1t8, "w2p": w2p, "w3p": w3p,
              "w4p": w4p, "b1p": b1p, "b2p": b2p, "b3p": b3p, "b4p": b4p,
              "selp": selp}

    in_maps = []
    for c in range(CORES):
        xc = x[c * BC:(c + 1) * BC]                         # [BC, DIN]
        hi = xc.astype(f16)
        lo = (xc.astype(np.float64) - hi.astype(np.float64))
        a16 = np.zeros((K16 * 128, BC), f16)
        a16[:DIN] = hi.T
        a16[DIN:DIN + 112] = lo.T[NLO:DIN].astype(f16)
        a8 = np.zeros((KD * 256, BC), np.float64)
        a8[:NLO] = lo.T[:NLO] * LSC
        xt16c = np.ascontiguousarray(
            a16.reshape(K16, 128, BC).transpose(1, 0, 2))
        xt8c = np.ascontiguousarray(
            a8.reshape(KD, 2, 128, BC).transpose(2, 0, 1, 3)).astype(FP8)
        in_maps.append({"xt16": xt16c, "xt8": xt8c, **shared})
    return in_maps


_cached_nc = None


def kernel(x, w1, b1, w2, b2, w3, b3, w4, b4):
    global _cached_nc, last_exec_time_ns
    import os
    trace = bool(int(os.environ.get("KERNEL_TRACE", "0")))
    if _cached_nc is None:
        _cached_nc = _build_program()
    in_maps = _pack_inputs(x, w1, b1, w2, b2, w3, b3, w4, b4)
    res = run_bass_kernel_spmd(_cached_nc, in_maps, list(range(CORES)),
                               trace=trace)
    last_exec_time_ns = res.exec_time_ns
    return np.concatenate([res.results[c]["out"] for c in range(CORES)], axis=0)


# revision 18
# speedup vs baseline: 1.0082x; 1.0001x over previous
"""Binarized 4-layer MLP (8192x784 -> 6144 -> 6144 -> 6144 -> 10, log_softmax)
on 8 Trainium2 NeuronCores, data-parallel over the batch.

Per-core dataflow (batch slice of 1024, feature-major activations [feat, batch]):
  fc1: x @ sign(w1).T as a hybrid split of x: hi = fp16(x) plus 112 exact fp16
       lo rows in the 7th k-tile's padding, and the remaining 672 lo rows
       (lo = x - hi) scaled by 2^9 in fp8e4 via 3 DoubleRow blocks whose
       stationary weights are +-2^-9 (exactly representable; PE handles fp8
       subnormal weights losslessly - HW verified). All terms accumulate into
       one PSUM group, so fc1 costs 10 passes instead of 13 at ~2^-15
       relative x error, which flips only ~1e-4 of h1 signs.
  fc2/fc3: sign(h) @ sign(w).T in fp8e4 with DoubleRow perf mode (fc2 uses
       the SwInterleave stationary layout; measured identical to DoubleRow).
       All products are +-1 and partial sums are small integers, so fp32 PSUM
       accumulation is bit-exact regardless of order.
  fc4: fused into the fc3 m-loop, single fp16 pass (w4 and h3 in fp16).
  log_softmax: PE-transpose of the logits to [batch, 10] tiles, exp/sum/ln
       without max-subtraction (logits are O(1), no overflow risk).

Schedule notes:
  - startup DMAs are split small and spread over the sync/gpsimd (+scalar
    early) DGE queues in consumption order
  - h1/h2 are split into lo/hi tiles so the next layer's first matmul
    doesn't wait on the last sign() of the previous layer
  - fc3/fc4 run per batch-half; the softmax tail of half 0 hides under the
    fc3 matmuls of half 1; one Ln at the very end serves both halves
"""

import numpy as np
import ml_dtypes

import concourse.bass as bass
import concourse.mybir as mybir
from concourse import bacc
from concourse.tile import TileContext
from concourse.bass_utils import run_bass_kernel_spmd
from concourse.masks import make_identity

dt = mybir.dt

CORES = 8
B = 8192
BC = B // CORES          # 1024 batch rows per core
DIN = 784
K16 = 7                  # fc1 fp16 k-tiles (784 hi + 112 exact lo rows)
KD = 3                   # fc1 fp8 DoubleRow blocks (672 lo rows + 96 pad)
NLO = 672                # lo rows carried in fp8
LSC = 512.0              # lo scale 2^9 (weights +-2^-9)
DH = 6144
MT = DH // 128           # 48 feature tiles
MH = MT // 2             # 24 tiles per lo/hi activation buffer
KB = DH // 256           # 24 DoubleRow contraction blocks
DOUT = 10
NH = BC // 512           # 2 moving halves of 512
NJ2 = 512 // 128         # 4 output j-tiles per half
MQ = 12                  # fc1 m-groups (w1 streamed per 4 m-tiles)
MPQ = MT // MQ

BF16 = ml_dtypes.bfloat16
FP8 = mybir.dt.np(dt.float8e4)

last_exec_time_ns = None


def _build_program():
    nc = bacc.Bacc("TRN2", target_bir_lowering=False, debug=False,
                   num_devices=CORES)

    xt16 = nc.dram_tensor("xt16", [128, K16, BC], dt.float16,
                          kind="ExternalInput").ap()
    xt8 = nc.dram_tensor("xt8", [128, KD, 2, BC], dt.float8e4,
                         kind="ExternalInput").ap()
    w1t16 = nc.dram_tensor("w1t16", [MQ, 128, K16, MPQ * 128], dt.float16,
                           kind="ExternalInput").ap()
    w1t8 = nc.dram_tensor("w1t8", [MQ, 128, KD, 2, MPQ * 128], dt.float8e4,
                          kind="ExternalInput").ap()
    w2p = nc.dram_tensor("w2p", [MT, 128, KB, 256], dt.float8e4,
                         kind="ExternalInput").ap()
    w3p = nc.dram_tensor("w3p", [MT, 128, KB, 2, 128], dt.float8e4,
                         kind="ExternalInput").ap()
    w4p = nc.dram_tensor("w4p", [128, MT, DOUT], dt.float16,
                         kind="ExternalInput").ap()
    b1p = nc.dram_tensor("b1p", [128, MT], dt.float32, kind="ExternalInput").ap()
    b2p = nc.dram_tensor("b2p", [128, MT], dt.float32, kind="ExternalInput").ap()
    b3p = nc.dram_tensor("b3p", [128, MT], dt.float32, kind="ExternalInput").ap()
    b4p = nc.dram_tensor("b4p", [DOUT, 1], dt.float32, kind="ExternalInput").ap()
    selp = nc.dram_tensor("selp", [128, DOUT], dt.float32,
                          kind="ExternalInput").ap()
    out = nc.dram_tensor("out", [BC, DOUT], dt.float32, kind="ExternalOutput").ap()

    DR = mybir.MatmulPerfMode.DoubleRow
    DRSW = mybir.MatmulPerfMode.DoubleRowSwInterleave
    AF = mybir.ActivationFunctionType

    with TileContext(nc) as tc:
        with tc.tile_pool(name="consts", bufs=1) as cpool, \
             tc.tile_pool(name="h1p", bufs=1) as h1pool:
            h1 = [h1pool.tile([128, MH, BC], dt.float8e4, tag=f"h1{i}",
                              name=f"h1{i}") for i in range(2)]

            # prefetched first w2/w3 m-tiles (their zone opens mid-program)
            w2f = cpool.tile([128, KB, 256], dt.float8e4)
            w3f = cpool.tile([128, KB, 2, 128], dt.float8e4)
            b1_sb = cpool.tile([128, MT], dt.float32)
            b2_sb = cpool.tile([128, MT], dt.float32)
            b3_sb = cpool.tile([128, MT], dt.float32)
            b4_sb = cpool.tile([DOUT, 1], dt.float32)
            w4_sb = cpool.tile([128, MT, DOUT], dt.float16)
            ident = cpool.tile([DOUT, DOUT], dt.float32)
            sel_sb = cpool.tile([128, DOUT], dt.float32)

            # ---------------- fc1 ----------------
            with tc.tile_pool(name="fc1in", bufs=1) as fpool, \
                 tc.tile_pool(name="w1pool", bufs=3) as w1pool, \
                 tc.tile_pool(name="ps1", bufs=3, space="PSUM") as ps1:
                # startup DMAs, small pieces in consumption order; scalar's
                # DGE only helps before the ACT engine starts sign() work
                jobs = []
                xt_half = {}
                w1q0a = {}
                w1q0b = {}
                tiles = {}
                for k in range(K16):
                    txa = fpool.tile([128, 512], dt.float16, tag=f"xta_{k}",
                                     name=f"xta_{k}")
                    txb = fpool.tile([128, 512], dt.float16, tag=f"xtb_{k}",
                                     name=f"xtb_{k}")
                    twa = fpool.tile([128, 128], dt.float16, tag=f"w1a_{k}",
                                     name=f"w1a_{k}")
                    twb = fpool.tile([128, 384], dt.float16, tag=f"w1b_{k}",
                                     name=f"w1b_{k}")
                    tiles[k] = (txa, txb, twa, twb)
                    xt_half[(k, 0)] = txa[:, :]
                    xt_half[(k, 1)] = txb[:, :]
                    w1q0a[k] = twa
                    w1q0b[k] = twb
                x8t = fpool.tile([128, KD, 2, BC], dt.float8e4)
                w1q08 = fpool.tile([128, KD, 2, MPQ * 128], dt.float8e4)

                def kjobs(k):
                    txa, txb, twa, twb = tiles[k]
                    if k < 2:
                        # first tiles split across rings to halve the
                        # latency of the very first transfers
                        return [(twa[:, :], w1t16[0, :, k, 0:128]),
                                (txa[:, 0:256], xt16[:, k, 0:256]),
                                (txa[:, 256:512], xt16[:, k, 256:512]),
                                (txb[:, 0:256], xt16[:, k, 512:768]),
                                (txb[:, 256:512], xt16[:, k, 768:1024]),
                                (twb[:, :], w1t16[0, :, k, 128:512])]
                    return [(twa[:, :], w1t16[0, :, k, 0:128]),
                            (txa[:, :], xt16[:, k, 0:512]),
                            (txb[:, :], xt16[:, k, 512:1024]),
                            (twb[:, :], w1t16[0, :, k, 128:512])]

                # consumption order: k=0, then the sandwiched DR operands,
                # then k=1..6, then the m>0 parts of the q0 weights
                jobs += kjobs(0)
                jobs += [(w1q08[:, :, :, 0:128], w1t8[0, :, :, :, 0:128])]
                for b in range(KD):
                    jobs += [(x8t[:, b], xt8[:, b])]
                for k in range(1, K16):
                    jobs += kjobs(k)
                jobs += [(w1q08[:, :, :, 128:512], w1t8[0, :, :, :, 128:512])]
                for i, (dst, src) in enumerate(jobs):
                    q = ([nc.sync, nc.gpsimd, nc.scalar][i % 3] if i < 12
                         else [nc.sync, nc.gpsimd][i % 2])
                    q.dma_start(out=dst, in_=src)

                nc.sync.dma_start(out=b1_sb[:], in_=b1p[:])
                nc.gpsimd.dma_start(out=b2_sb[:], in_=b2p[:])
                nc.sync.dma_start(out=b3_sb[:], in_=b3p[:])
                nc.gpsimd.dma_start(out=b4_sb[:], in_=b4p[:])
                nc.sync.dma_start(out=w4_sb[:], in_=w4p[:])
                nc.gpsimd.dma_start(out=sel_sb[:], in_=selp[:])
                make_identity(nc, ident[:])

                for q in range(MQ):
                    if q == 0:
                        def lhs16(k, mi):
                            if mi == 0:
                                return w1q0a[k][:, :]
                            return w1q0b[k][:, (mi - 1) * 128:mi * 128]

                        def lhs8(b, mi):
                            return w1q08[:, b, :, mi * 128:(mi + 1) * 128]
                    else:
                        # q==1 must load during the startup crunch (sync);
                        # later groups dispatch from the scalar queue, which
                        # is backed up behind sign() ACTs — a free just-in-
                        # time delay that keeps these 1.3MB streams out of
                        # the startup DMA crunch
                        wq = nc.sync if q == 1 else nc.scalar
                        w1q16 = w1pool.tile([128, K16, MPQ * 128], dt.float16,
                                            tag="w1q16")
                        wq.dma_start(out=w1q16[:], in_=w1t16[q])
                        w1q8 = w1pool.tile([128, KD, 2, MPQ * 128],
                                           dt.float8e4, tag="w1q8")
                        (nc.gpsimd if q == 1 else nc.scalar).dma_start(
                            out=w1q8[:], in_=w1t8[q])

                        def lhs16(k, mi, w1q16=w1q16):
                            return w1q16[:, k, mi * 128:(mi + 1) * 128]

                        def lhs8(b, mi, w1q8=w1q8):
                            return w1q8[:, b, :, mi * 128:(mi + 1) * 128]
                    for mi in range(MPQ):
                        m = q * MPQ + mi
                        psum = ps1.tile([128, BC], dt.float32, tag="ps1")

                        def mm16(k, start, stop, mi=mi, psum=psum):
                            for n in range(NH):
                                nc.tensor.matmul(
                                    psum[:, n * 512:(n + 1) * 512],
                                    lhs16(k, mi), xt_half[(k, n)],
                                    start=start, stop=stop)

                        def mmdr(b, start, stop, mi=mi, psum=psum):
                            for n in range(NH):
                                nc.tensor.matmul(
                                    psum[:, n * 512:(n + 1) * 512],
                                    lhs8(b, mi),
                                    x8t[:, b, :, n * 512:(n + 1) * 512],
                                    start=start, stop=stop, perf_mode=DR)

                        # alternate group orientation so consecutive groups
                        # meet DR-to-DR at the boundary: one fp16<->DR mode
                        # switch (~100ns) per group instead of two
                        if m % 2 == 0:
                            for k in range(K16):
                                mm16(k, k == 0, False)
                            for b in range(KD):
                                mmdr(b, False, b == KD - 1)
                        else:
                            for b in range(KD):
                                mmdr(b, b == 0, False)
                            for k in range(K16):
                                mm16(k, False, k == K16 - 1)
                        nc.scalar.sign(h1[m // MH][:, m % MH, :], psum[:, :],
                                       bias=b1_sb[:, m:m + 1])
                        if m == 20:
                            # deferred w2/w3 first-tile prefetch: scalar's
                            # queue is behind ~20 signs, so these 786KB
                            # loads dispatch ~100us in, clear of the crunch
                            nc.scalar.dma_start(out=w2f[:], in_=w2p[0])
                        elif m == 21:
                            nc.scalar.dma_start(out=w3f[:], in_=w3p[0])

            # ---------------- fc2 ----------------
            def pair(h, b, n):
                # moving [128, 2, 512] for DR block b out of lo/hi buffers
                t = 2 * b
                return h[t // MH][:, t % MH:t % MH + 2, n * 512:(n + 1) * 512]

            with tc.tile_pool(name="h2p", bufs=1) as h2pool:
                h2 = [h2pool.tile([128, MH, BC], dt.float8e4, tag=f"h2{i}",
                                  name=f"h2{i}") for i in range(2)]
                with tc.tile_pool(name="w2pool", bufs=3) as w2pool, \
                     tc.tile_pool(name="ps2", bufs=3, space="PSUM") as ps2:
                    for m in range(MT):
                        if m == 0:
                            wsb = w2f
                        else:
                            wsb = w2pool.tile([128, KB, 256], dt.float8e4,
                                              tag="w2")
                            nc.sync.dma_start(out=wsb[:], in_=w2p[m])
                        psum = ps2.tile([128, BC], dt.float32, tag="ps2")
                        for n in range(NH):
                            for b in range(KB):
                                nc.tensor.matmul(
                                    psum[:, n * 512:(n + 1) * 512],
                                    wsb[:, b],
                                    pair(h1, b, n),
                                    start=(b == 0),
                                    stop=(b == KB - 1),
                                    perf_mode=DRSW,
                                )
                        nc.scalar.sign(h2[m // MH][:, m % MH, :], psum[:, :],
                                       bias=b2_sb[:, m:m + 1])

                # ------------- fc3 + fused fc4 + log_softmax -------------
                with tc.tile_pool(name="lgp", bufs=1, space="PSUM") as lgp, \
                     tc.tile_pool(name="zsp", bufs=1, space="PSUM") as zsp, \
                     tc.tile_pool(name="tpp", bufs=2, space="PSUM") as tpp, \
                     tc.tile_pool(name="smp", bufs=1) as smp, \
                     tc.tile_pool(name="w3pool", bufs=3) as w3pool, \
                     tc.tile_pool(name="h3pool", bufs=34) as h3pool, \
                     tc.tile_pool(name="ps3", bufs=3, space="PSUM") as ps3:
                    zex = smp.tile([128, NH * NJ2, DOUT], dt.float32)
                    zlog = smp.tile([128, NH * NJ2, DOUT], dt.float32)
                    lg_psums = {}
                    lg_sbs = {}

                    def tail_head(n):
                        lgcp = smp.tile([128, 512], dt.float32,
                                        tag="lgcp", name="lgcp")
                        nc.scalar.copy(lgcp[:], lg_psums[n][:])
                        zsum = zsp.tile([DOUT, 512], dt.float32, tag="zs",
                                        name="zs")
                        nc.tensor.matmul(zsum[:], sel_sb[:], lgcp[:],
                                         start=True, stop=True)
                        lg_sb = smp.tile([DOUT, 512], dt.float32,
                                         tag=f"lgsb{n}", name=f"lgsb{n}")
                        nc.scalar.activation(lg_sb[:], zsum[:],
                                             AF.Identity, bias=b4_sb[:, 0:1])
                        lg_sbs[n] = lg_sb

                    def tail_j(n, j):
                        t = n * NJ2 + j
                        tp = tpp.tile([128, DOUT], dt.float32, tag="tp",
                                      name="tp")
                        nc.tensor.transpose(
                            tp[:], lg_sbs[n][:, j * 128:(j + 1) * 128],
                            ident[:])
                        nc.scalar.activation(zex[:, t, :], tp[:], AF.Exp)
                        nc.vector.tensor_scalar_add(zlog[:, t, :], tp[:], 0.0)

                    for n in range(NH):
                        lg_psum = lgp.tile([128, 512], dt.float32,
                                           tag=f"lg{n}", name=f"lg{n}")
                        lg_psums[n] = lg_psum
                        h3_tiles = [None] * MT

                        def fc4_mm(m, lg_psum=lg_psum, h3_tiles=h3_tiles):
                            # 4 interleaved accumulators in distinct 32-col
                            # PE subarrays: quads of these run concurrently
                            g = m % 4
                            nc.tensor.matmul(
                                lg_psum[32 * g:32 * g + DOUT, :],
                                w4_sb[:, m, :],
                                h3_tiles[m][:, :],
                                start=(m < 4),
                                stop=(m >= MT - 4),
                                tile_position=(0, 32 * g),
                                skip_group_check=True,
                            )

                        for m in range(MT):
                            if n == 0 and m == 0:
                                wsb = w3f
                            else:
                                wsb = w3pool.tile([128, KB, 2, 128],
                                                  dt.float8e4, tag="w3")
                                (nc.sync if m % 2 else nc.gpsimd).dma_start(
                                    out=wsb[:], in_=w3p[m])
                            psum = ps3.tile([128, 512], dt.float32, tag="ps3")
                            for b in range(KB):
                                nc.tensor.matmul(
                                    psum[:, :],
                                    wsb[:, b],
                                    pair(h2, b, n),
                                    start=(b == 0),
                                    stop=(b == KB - 1),
                                    perf_mode=DR,
                                )
                            t_h3 = h3pool.tile([128, 512], dt.float16,
                                               tag="h3")
                            nc.scalar.activation(t_h3[:], psum[:, :],
                                                 AF.Identity,
                                                 bias=b3_sb[:, m:m + 1])
                            nc.vector.tensor_scalar(
                                t_h3[:], t_h3[:], 1.0, -1.0,
                                mybir.AluOpType.min, mybir.AluOpType.max)
                            h3_tiles[m] = t_h3
                            # fc4 batched every 8 m-tiles, one group behind
                            # so the PE never waits on ACT/DVE
                            if m % 16 == 15 and m >= 31:
                                for mm in range(m - 31, m - 15):
                                    fc4_mm(mm)
                            # half-0 softmax tail hides under half-1 fc3
                            if n == 1:
                                if m == 2:
                                    tail_head(0)
                                elif 4 <= m <= 7:
                                    tail_j(0, m - 4)
                        for mm in range(MT - 16, MT):
                            fc4_mm(mm)

                    # ------------- final softmax tail (half 1) -------------
                    tail_head(1)
                    for j in range(NJ2):
                        tail_j(1, j)
                    sums = smp.tile([128, NH * NJ2], dt.float32, tag="sums")
                    nc.vector.tensor_reduce(sums[:], zex[:, :, :],
                                            mybir.AxisListType.X,
                                            mybir.AluOpType.add)
                    lns = smp.tile([128, NH * NJ2], dt.float32, tag="lns")
                    nc.scalar.activation(lns[:], sums[:], AF.Ln)
                    for t in range(NH * NJ2):
                        res = smp.tile([128, DOUT], dt.float32, tag=f"res{t}",
                                       name=f"res{t}")
                        nc.vector.tensor_scalar(res[:], zlog[:, t, :],
                                                lns[:, t:t + 1], None,
                                                mybir.AluOpType.subtract)
                        nc.sync.dma_start(
                            out=out[t * 128:(t + 1) * 128, :], in_=res[:])

    nc.compile()
    return nc


def _pack_inputs(x, w1, b1, w2, b2, w3, b3, w4, b4):
    """Host-side packing into the device layouts. Shared tensors are packed
    once; only xt16/xt8 differ per core."""
    f32 = np.float32
    f16 = np.float16
    x = np.asarray(x, f32).reshape(B, DIN)

    s1 = np.sign(np.asarray(w1, f32))                       # [DH, DIN]
    # fp16 stationary stack: 784 hi rows + 112 lo rows (features 672..783)
    s16 = np.zeros((K16 * 128, DH), f16)
    s16[:DIN] = s1.T
    s16[DIN:DIN + 112] = s1.T[NLO:DIN]
    w1t16 = np.ascontiguousarray(
        s16.reshape(K16, 128, MQ, MPQ * 128).transpose(2, 1, 0, 3))
    # fp8 stationary: +-2^-9 for lo features 0..671, zero-padded to 768
    s8 = np.zeros((KD * 256, DH), f32)
    s8[:NLO] = s1.T[:NLO] / LSC
    w1t8 = np.ascontiguousarray(
        s8.reshape(KD, 2, 128, MQ, MPQ * 128).transpose(3, 2, 0, 1, 4)
    ).astype(FP8)

    def pack_dr(w):
        # sign(w).T -> [mo, p, b, i, m'] DoubleRow stationary layout
        st = np.sign(np.asarray(w, f32)).T                  # [in, out]
        r = st.reshape(KB, 2, 128, MT, 128)                 # [b, i, p, mo, m']
        return np.ascontiguousarray(r.transpose(3, 2, 0, 1, 4)).astype(FP8)

    def pack_dr_swi(w):
        # [mo, p, b, 2*(127-m')+i] (A/B interleaved, reversed columns)
        st = np.sign(np.asarray(w, f32)).T                  # [in, out]
        r = st.reshape(KB, 2, 128, MT, 128)                 # [b, i, p, mo, m']
        t5 = r.transpose(3, 2, 0, 1, 4)                     # [mo, p, b, i, m']
        return np.ascontiguousarray(
            t5[:, :, :, :, ::-1].transpose(0, 1, 2, 4, 3)
            .reshape(MT, 128, KB, 256)).astype(FP8)

    w2p = pack_dr_swi(w2)
    w3p = pack_dr(w3)

    # fc4 weights: w4.T in fp16, layout [p, j, c]
    w4t = np.asarray(w4, f32).T.astype(f16)                 # [DH, DOUT]
    w4p = np.ascontiguousarray(w4t.reshape(MT, 128, DOUT).transpose(1, 0, 2))

    def pack_b(b):
        return np.ascontiguousarray(np.asarray(b, f32).reshape(MT, 128).T)

    b1p, b2p, b3p = pack_b(b1), pack_b(b2), pack_b(b3)
    b4p = np.asarray(b4, f32).reshape(DOUT, 1)
    selp = np.zeros((128, DOUT), f32)
    for g in range(4):
        selp[32 * g + np.arange(DOUT), np.arange(DOUT)] = 1.0

    shared = {"w1t16": w1t16, "w1t8": w1t8, "w2p": w2p, "w3p": w3p,
              "w4p": w4p, "b1p": b1p, "b2p": b2p, "b3p": b3p, "b4p": b4p,
              "selp": selp}

    in_maps = []
    for c in range(CORES):
        xc = x[c * BC:(c + 1) * BC]                         # [BC, DIN]
        hi = xc.astype(f16)
        lo = (xc.astype(np.float64) - hi.astype(np.float64))
        a16 = np.zeros((K16 * 128, BC), f16)
        a16[:DIN] = hi.T
        a16[DIN:DIN + 112] = lo.T[NLO:DIN].astype(f16)
        a8 = np.zeros((KD * 256, BC), np.float64)
        a8[:NLO] = lo.T[:NLO] * LSC
        xt16c = np.ascontiguousarray(
            a16.reshape(K16, 128, BC).transpose(1, 0, 2))
        xt8c = np.ascontiguousarray(
            a8.reshape(KD, 2, 128, BC).transpose(2, 0, 1, 3)).astype(FP8)
        in_maps.append({"xt16": xt16c, "xt8": xt8c, **shared})
    return in_maps


_cached_nc = None


def kernel(x, w1, b1, w2, b2, w3, b3, w4, b4):
    global _cached_nc, last_exec_time_ns
    import os
    trace = bool(int(os.environ.get("KERNEL_TRACE", "0")))
    if _cached_nc is None:
        _cached_nc = _build_program()
    in_maps = _pack_inputs(x, w1, b1, w2, b2, w3, b3, w4, b4)
    res = run_bass_kernel_spmd(_cached_nc, in_maps, list(range(CORES)),
                               trace=trace)
    last_exec_time_ns = res.exec_time_ns
    return np.concatenate([res.results[c]["out"] for c in range(CORES)], axis=0)
